# revision 21
# baseline (speedup 1.0000x reference)
"""GQA attention layer (B=2, S=2048, HID=4096, 32 Q heads / 8 KV heads, RoPE,
causal) on 8 TRN2 NeuronCores.

Strategy (tensor-parallel over heads):
  - core c owns Q heads 4c..4c+3 and KV head c (one full GQA group).
  - host pre-transposes x and weights into [128, kc, feat] layouts so every
    input loads with a handful of large DMAs and every on-chip matmul
    contracts over the partition axis with no on-chip transposes of x.
  - Q/K/V are SBUF-resident: projections write Q^T/K^T (RoPE fused into the
    PSUM eviction on DVE) and V (TensorE-transposed) straight into
    persistent SBUF tiles; attention reads them with zero DMA.
  - the PE stream is kept dense end-to-end: batch-0 attention heads are
    interleaved with the batch-1 projection groups (so the AllGather chain
    starts ~half-way into the projection phase), and batch-1 attention
    heads are interleaved with o_proj chunks of already-gathered blocks.
  - attention is emitted score-lookahead (s0,s1,c0,s2,c1,...) so the PE
    FIFO never waits on the ACT exp; causal diagonal 128-chunks use
    narrowed q-ranges (exact triangle FLOPs) with a single 128x128
    triangular mask multiply; softmax denominator accumulates in bf16 and
    becomes one ones[128x128] matmul broadcast + ACT copy + DVE divide.
  - ctx^T is AllGathered per 512-token block (8 chunks); the host
    concatenates the 8 per-core 512-column o_proj output slices.
"""

import os

os.environ.setdefault("NEURON_RT_DBG_RDH_CC", "0")

import numpy as np
import ml_dtypes

B, S, HID = 2, 2048, 4096
NH, NKV, D = 32, 8, 128
T = B * S            # 4096 flattened tokens
NQ = 512             # per-core q features (4 heads x 128)
P = 128
TOKB = 512           # token block (matmul moving free dim)
NB = T // TOKB       # 8 token blocks
KC = HID // P        # 32 contraction chunks for projections
QBS = S // TOKB      # 4 q blocks per batch
KTS = S // P         # 16 k chunks per batch
NHL = 4              # local Q heads per core
SCALE = 1.0 / float(np.sqrt(np.float32(D)))
N_CORES = 8

_BUILT = None
LAST_RESULTS = None


def _build():
    from contextlib import ExitStack

    import concourse.tile as tile
    from concourse import bacc, mybir

    f32 = mybir.dt.float32
    bf16 = mybir.dt.bfloat16
    Exp = mybir.ActivationFunctionType.Exp

    nc = bacc.Bacc(
        "TRN2",
        target_bir_lowering=False,
        debug=False,
        num_devices=N_CORES,
    )

    xT = nc.declare_dram_parameter("xT", [P, KC, T], bf16, isOutput=False)
    wqT = nc.declare_dram_parameter("wqT", [P, KC, NQ], bf16, isOutput=False)
    wkT = nc.declare_dram_parameter("wkT", [P, KC, D], bf16, isOutput=False)
    wvT = nc.declare_dram_parameter("wvT", [P, KC, D], bf16, isOutput=False)
    woT = nc.declare_dram_parameter("woT", [P, KC, NQ], bf16, isOutput=False)
    cosT = nc.declare_dram_parameter("cosT", [64, T], f32, isOutput=False)
    sinT = nc.declare_dram_parameter("sinT", [64, T], f32, isOutput=False)
    maskT = nc.declare_dram_parameter("maskT", [P, P], bf16, isOutput=False)
    outT = nc.declare_dram_parameter("outT", [NQ, T], f32, isOutput=True)

    XC = 8               # kc chunks per x DMA
    NXD = KC // XC       # 4 x DMAs per token block

    with tile.TileContext(nc) as tc, ExitStack() as gctx:
        ec = gctx.enter_context
        # ---- global pools (whole-kernel lifetime) ----
        dram = ec(tc.tile_pool(name="dram", bufs=1, space="DRAM"))
        const_pool = ec(tc.tile_pool(name="const_sb", bufs=1))
        qkv_pool = ec(tc.tile_pool(name="qkv_sb", bufs=1))
        # PSUM budget (8 banks): ps 3 + ctxp 2 + rsp {vps,rbp} 2 + opp 1
        ps_pool = ec(tc.tile_pool(name="ps", bufs=3, space="PSUM"))
        ctxp_pool = ec(tc.tile_pool(name="ctxp", bufs=2, space="PSUM"))
        rs_pool = ec(tc.tile_pool(name="rsp", bufs=1, space="PSUM"))
        op_pool = ec(tc.tile_pool(name="opp", bufs=1, space="PSUM"))
        # attention working pools (live through both halves)
        e_pool = ec(tc.tile_pool(name="e_sb", bufs=6))
        acc_pool = ec(tc.tile_pool(name="acc_sb", bufs=3))
        rbc_pool = ec(tc.tile_pool(name="rbc_sb", bufs=2))
        ctx_out_pool = ec(tc.tile_pool(name="ctx_sb", bufs=2))

        # token-chunked AllGather buffers (0.5 MB per rank per chunk)
        ag_in = [dram.tile([P, NHL * TOKB], bf16, name=f"ag_in{t}")
                 for t in range(NB)]
        ag_out = [
            dram.tile([N_CORES * P, NHL * TOKB], bf16, addr_space="Shared",
                      name=f"ag_out{t}")
            for t in range(NB)
        ]

        ones_sq = const_pool.tile([P, P], bf16, name="ones_sq")
        nc.vector.memset(ones_sq[:, :], 1.0)
        ident = const_pool.tile([P, P], bf16, name="ident")
        from concourse.masks import make_identity
        make_identity(nc, ident[:, :])
        tri_sb = const_pool.tile([P, P], bf16, name="tri_sb")
        nc.sync.dma_start(out=tri_sb[:, :], in_=maskT[:, :])

        # persistent Q^T / K^T / V tiles (SBUF-resident between phases)
        q_sb = qkv_pool.tile([P, NHL, T], bf16, name="q_sb")
        kt_sb = qkv_pool.tile([P, T], bf16, name="kt_sb")
        v_sb = [qkv_pool.tile([P, P], bf16, name=f"v_sb{g}")
                for g in range(T // P)]  # 32 chunks

        # ---------------- attention head / AG emitters ----------------
        def attn_head(tb, h, ctxw):
            b, qb = tb // QBS, tb % QBS
            qcols = slice(S * b + TOKB * qb, S * b + TOKB * (qb + 1))
            nkt = 4 * qb + 4
            qh = q_sb[:, h, qcols]
            acc = acc_pool.tile([P, TOKB], bf16, name="acc")
            ctxp = ctxp_pool.tile([P, TOKB], f32, name="ctxp")
            es = [None] * nkt

            def q_lo(kt):
                j = kt - 4 * qb
                return 0 if j < 0 else P * j

            def emit_score(kt):
                lo = q_lo(kt)
                sp = ps_pool.tile([P, TOKB], f32, name="ps")
                nc.tensor.matmul(
                    sp[:, lo:],
                    kt_sb[:, S * b + P * kt:S * b + P * (kt + 1)],
                    qh[:, lo:],
                    start=True, stop=True,
                )
                e = e_pool.tile([P, TOKB], bf16, name="e")
                nc.scalar.activation(e[:, lo:], sp[:, lo:], Exp, scale=SCALE)
                if kt - 4 * qb >= 0:
                    nc.vector.tensor_mul(
                        e[:, lo:lo + P], e[:, lo:lo + P], tri_sb[:, :]
                    )
                if kt == 0:
                    nc.vector.tensor_copy(acc[:, lo:], e[:, lo:])
                else:
                    nc.vector.tensor_add(acc[:, lo:], acc[:, lo:], e[:, lo:])
                es[kt] = e

            def emit_ctx(kt):
                lo = q_lo(kt)
                nc.tensor.matmul(
                    ctxp[:, lo:],
                    v_sb[16 * b + kt][:, :],
                    es[kt][:, lo:],
                    start=(kt == 0), stop=(kt == nkt - 1),
                )

            # score-lookahead emission: s0 s1 c0 s2 c1 ... c(n-1)
            emit_score(0)
            for kt in range(1, nkt):
                emit_score(kt)
                emit_ctx(kt - 1)
            emit_ctx(nkt - 1)

            # denominator: ones[128,128]^T @ acc = broadcast rowsum
            rbp = rs_pool.tile([P, TOKB], f32, name="rbp")
            nc.tensor.matmul(
                rbp[:, :], ones_sq[:, :], acc[:, :], start=True, stop=True
            )
            rbc = rbc_pool.tile([P, TOKB], f32, name="rbc")
            nc.vector.reciprocal(rbc[:, :], rbp[:, :])
            nc.vector.tensor_mul(
                ctxw[:, TOKB * h:TOKB * (h + 1)], ctxp[:, :], rbc[:, :]
            )
            # per-head store so the AllGather trigger only waits on head 3
            nc.sync.dma_start(
                out=ag_in[tb][:, TOKB * h:TOKB * (h + 1)],
                in_=ctxw[:, TOKB * h:TOKB * (h + 1)],
            )

        def emit_ag(tb, ctxw):
            nc.gpsimd.collective_compute(
                "AllGather",
                mybir.AluOpType.bypass,
                replica_groups=[list(range(N_CORES))],
                ins=[ag_in[tb][:, :].opt()],
                outs=[ag_out[tb][:, :].opt()],
            )

        # ================= Projections + batch-0 attention =================
        with ExitStack() as actx:
            aec = actx.enter_context
            w_pool = aec(tc.tile_pool(name="w_sb", bufs=1))
            xt_pool = aec(tc.tile_pool(name="xt_sb", bufs=5))
            rope_pool = aec(tc.tile_pool(name="rope_sb", bufs=1))
            rtmp_pool = aec(tc.tile_pool(name="rtmp_sb", bufs=2))
            vtmp_pool = aec(tc.tile_pool(name="vtmp_sb", bufs=2))

            # wq + x(block 0) first in the DMA queue: they gate the first MM
            wq_sb = w_pool.tile([P, KC, NQ], bf16, name="wq")
            nc.sync.dma_start(out=wq_sb[:, :, :], in_=wqT[:, :, :])
            wk_sb = w_pool.tile([P, KC, D], bf16, name="wk")
            wv_sb = w_pool.tile([P, KC, D], bf16, name="wv")
            cos_sb = rope_pool.tile([64, T], f32, name="cos_sb")
            sin_sb = rope_pool.tile([64, T], f32, name="sin_sb")

            def load_rest():
                nc.sync.dma_start(out=wk_sb[:, :, :], in_=wkT[:, :, :])
                nc.sync.dma_start(out=wv_sb[:, :, :], in_=wvT[:, :, :])
                nc.sync.dma_start(out=cos_sb[:, :], in_=cosT[:, :])
                nc.sync.dma_start(out=sin_sb[:, :], in_=sinT[:, :])

            def rope_evict(psum, dest, cols):
                """psum [128(d), 512(tok)] f32 -> RoPE -> dest bf16 slices."""
                c_ap = cos_sb[:, cols]
                s_ap = sin_sb[:, cols]
                p0 = psum[0:64, :]
                p1 = psum[64:128, :]
                t0 = rtmp_pool.tile([64, TOKB], f32, name="t0")
                t1 = rtmp_pool.tile([64, TOKB], f32, name="t1")
                nc.vector.tensor_mul(t0[:, :], p0, c_ap)
                nc.vector.tensor_mul(t1[:, :], p1, s_ap)
                nc.vector.tensor_sub(dest[0], t0[:, :], t1[:, :])
                t2 = rtmp_pool.tile([64, TOKB], f32, name="t2")
                t3 = rtmp_pool.tile([64, TOKB], f32, name="t3")
                nc.vector.tensor_mul(t2[:, :], p0, s_ap)
                nc.vector.tensor_mul(t3[:, :], p1, c_ap)
                nc.vector.tensor_add(dest[1], t2[:, :], t3[:, :])

            def proj_thunks(nb):
                """6 emission thunks for token block nb; thunk 0 also emits
                the x DMAs."""
                cols = slice(TOKB * nb, TOKB * (nb + 1))
                xts = []

                def load_x():
                    for xd in range(NXD):
                        xt_t = xt_pool.tile([P, XC, TOKB], bf16, name="xt")
                        nc.sync.dma_start(
                            out=xt_t[:, :, :],
                            in_=xT[:, XC * xd:XC * (xd + 1), cols],
                        )
                        xts.append(xt_t)

                def xr(kc):
                    return xts[kc // XC][:, kc % XC, :]

                def q_group(m):
                    psum = ps_pool.tile([P, TOKB], f32, name="ps")
                    for kc in range(KC):
                        nc.tensor.matmul(
                            psum[:, :],
                            wq_sb[:, kc, P * m:P * (m + 1)],
                            xr(kc),
                            start=(kc == 0), stop=(kc == KC - 1),
                        )
                    rope_evict(
                        psum,
                        (q_sb[0:64, m, cols], q_sb[64:128, m, cols]),
                        cols,
                    )

                def k_group():
                    psum = ps_pool.tile([P, TOKB], f32, name="ps")
                    for kc in range(KC):
                        nc.tensor.matmul(
                            psum[:, :], wk_sb[:, kc, :], xr(kc),
                            start=(kc == 0), stop=(kc == KC - 1),
                        )
                    rope_evict(
                        psum, (kt_sb[0:64, cols], kt_sb[64:128, cols]), cols
                    )

                def v_group():
                    psum = ps_pool.tile([P, TOKB], f32, name="ps")
                    for kc in range(KC):
                        nc.tensor.matmul(
                            psum[:, :], wv_sb[:, kc, :], xr(kc),
                            start=(kc == 0), stop=(kc == KC - 1),
                        )
                    vtmp = vtmp_pool.tile([P, TOKB], bf16, name="vtmp")
                    nc.scalar.copy(vtmp[:, :], psum[:, :])
                    for c in range(4):
                        vps = rs_pool.tile([P, P], bf16, name="vps")
                        nc.tensor.transpose(
                            vps[:, :], vtmp[:, P * c:P * (c + 1)], ident[:, :]
                        )
                        nc.scalar.copy(v_sb[4 * nb + c][:, :], vps[:, :])

                return ([load_x] + [lambda m=m: q_group(m) for m in range(4)]
                        + [k_group, v_group])

            # part 1: projections for batch 0 (blocks 0..3), dense.
            # x(0) DMA right after wq; remaining weights + rope tables after.
            for nb in range(4):
                ths = proj_thunks(nb)
                ths[0]()
                if nb == 0:
                    load_rest()
                for th in ths[1:]:
                    th()

            # part 2: batch-0 attention interleaved with batch-1 projections
            filler = []
            for nb in range(4, 8):
                filler.extend(proj_thunks(nb))
            pops = [2] * 12 + [1] * 4  # 28 thunks over 16 head slots
            slot = 0
            for tb in range(4):
                ctxw = ctx_out_pool.tile([P, NHL * TOKB], bf16, name="ctxw")
                for h in range(NHL):
                    attn_head(tb, h, ctxw)
                    for _ in range(pops[slot]):
                        if filler:
                            filler.pop(0)()
                    slot += 1
                emit_ag(tb, ctxw)
            while filler:
                filler.pop(0)()

        # ============ batch-1 attention + interleaved o_proj ============
        with ExitStack() as bctx:
            bec = bctx.enter_context
            wo_pool = bec(tc.tile_pool(name="wo_sb", bufs=1))
            cx_pool = bec(tc.tile_pool(name="cx_sb", bufs=2))
            o_out_pool = bec(tc.tile_pool(name="o_sb", bufs=3))

            wo_sb = wo_pool.tile([P, KC, NQ], bf16, name="wo")
            nc.sync.dma_start(out=wo_sb[:, :, :], in_=woT[:, :, :])

            cx_tiles = {}

            def load_cx(t):
                # ag_out rows 128r+d; SBUF chunk index fc = 4r+h matches the
                # natural global-head feature order of woT.
                cx = cx_pool.tile([P, N_CORES, NHL, TOKB], bf16, name="cx")
                nc.sync.dma_start(
                    out=cx[:, :, :, :],
                    in_=ag_out[t][:, :].rearrange(
                        "(r d) (h c) -> d r h c", r=N_CORES, h=NHL
                    ),
                )
                cx_tiles[t] = cx

            def oproj_chunk(t, ot):
                cols = slice(TOKB * t, TOKB * (t + 1))
                cx = cx_tiles[t]
                psum = op_pool.tile([P, TOKB], f32, name="ops")
                for fc in range(KC):
                    nc.tensor.matmul(
                        psum[:, :],
                        wo_sb[:, fc, P * ot:P * (ot + 1)],
                        cx[:, fc // NHL, fc % NHL, :],
                        start=(fc == 0), stop=(fc == KC - 1),
                    )
                ov = o_out_pool.tile([P, TOKB], f32, name="ov")
                nc.vector.tensor_copy(ov[:, :], psum[:, :])
                nc.sync.dma_start(
                    out=outT[P * ot:P * (ot + 1), cols], in_=ov[:, :]
                )

            load_cx(0)
            load_cx(1)
            opq = [(t, ot) for t in range(NB - 2) for ot in range(NHL)]
            pops2 = [1, 1, 2, 2, 2, 2, 2, 2, 2, 2, 1, 1, 1, 1, 1, 1]
            slot = 0
            for tb in range(4, NB):
                ctxw = ctx_out_pool.tile([P, NHL * TOKB], bf16, name="ctxw")
                for h in range(NHL):
                    attn_head(tb, h, ctxw)
                    for _ in range(pops2[slot]):
                        if opq:
                            t, ot = opq.pop(0)
                            if t not in cx_tiles:
                                load_cx(t)
                            oproj_chunk(t, ot)
                    slot += 1
                emit_ag(tb, ctxw)
            while opq:
                t, ot = opq.pop(0)
                if t not in cx_tiles:
                    load_cx(t)
                oproj_chunk(t, ot)

            load_cx(NB - 2)
            for ot in range(NHL):
                oproj_chunk(NB - 2, ot)

            # last block: stream cx in rank-pair quarters, accumulate the 4
            # output tiles in parallel so o_proj overlaps the cx DMAs
            t = NB - 1
            cols = slice(TOKB * t, TOKB * (t + 1))
            cxl = cx_pool.tile([P, N_CORES, NHL, TOKB], bf16, name="cx")
            for rr in range(4):
                nc.sync.dma_start(
                    out=cxl[:, 2 * rr:2 * (rr + 1), :, :],
                    in_=ag_out[t][2 * rr * P:2 * (rr + 1) * P, :].rearrange(
                        "(r d) (h c) -> d r h c", r=2, h=NHL
                    ),
                )
            psums = [op_pool.tile([P, TOKB], f32, name="ops")] + [
                ps_pool.tile([P, TOKB], f32, name="ps") for _ in range(3)
            ]
            for rr in range(4):
                for ot in range(NHL):
                    for fi in range(8):
                        fc = 8 * rr + fi
                        nc.tensor.matmul(
                            psums[ot][:, :],
                            wo_sb[:, fc, P * ot:P * (ot + 1)],
                            cxl[:, fc // NHL, fc % NHL, :],
                            start=(rr == 0 and fi == 0),
                            stop=(rr == 3 and fi == 7),
                        )
            for ot in range(NHL):
                ov = o_out_pool.tile([P, TOKB], f32, name="ov")
                nc.vector.tensor_copy(ov[:, :], psums[ot][:, :])
                nc.sync.dma_start(
                    out=outT[P * ot:P * (ot + 1), cols], in_=ov[:, :]
                )

    nc.finalize()
    return nc


def _host_prep(x, positions, w_q, w_k, w_v, w_o):
    bf = ml_dtypes.bfloat16

    def feat_major(w):
        # [F, HID] -> [128, KC, F]  (hid = 128*kc + p)
        F = w.shape[0]
        return np.ascontiguousarray(
            w.T.reshape(KC, P, F).transpose(1, 0, 2)
        ).astype(bf)

    xT = feat_major(x.reshape(T, HID))          # [128, 32, T]

    half = D // 2
    inv_freq = 1.0 / (10000.0 ** (np.arange(half, dtype=np.float32) / half))
    freqs = np.outer(np.asarray(positions, np.float32), inv_freq)  # [S, 64]
    cosT1 = np.cos(freqs).T.astype(np.float32)  # [64, S]
    sinT1 = np.sin(freqs).T.astype(np.float32)
    cosT = np.ascontiguousarray(np.concatenate([cosT1] * B, axis=1))
    sinT = np.ascontiguousarray(np.concatenate([sinT1] * B, axis=1))

    dk = np.arange(P, dtype=np.int64)[:, None]
    dq = np.arange(P, dtype=np.int64)[None, :]
    maskT = np.ascontiguousarray((dk <= dq).astype(np.float32)).astype(bf)

    in_maps = []
    for c in range(N_CORES):
        in_maps.append({
            "xT": xT,
            "wqT": feat_major(w_q[NQ * c:NQ * (c + 1), :]),
            "wkT": feat_major(w_k[D * c:D * (c + 1), :]),
            "wvT": feat_major(w_v[D * c:D * (c + 1), :]),
            "woT": feat_major(w_o[NQ * c:NQ * (c + 1), :]),
            "cosT": cosT, "sinT": sinT, "maskT": maskT,
        })
    return in_maps


def _ensure_ntff_hook():
    """The agent image's antenv lacks axon_hooks; synthesize it so
    run_bass_kernel_spmd(trace=True) can capture NTFF profiles."""
    import sys
    import types
    try:
        from antenv.axon_hooks import get_axon_ntff_profile_hook  # noqa: F401
        return
    except ImportError:
        pass
    import antenv
    mod = types.ModuleType("antenv.axon_hooks")
    _h = [None]
    mod.set_axon_ntff_profile_hook = lambda h: _h.__setitem__(0, h)
    mod.get_axon_ntff_profile_hook = lambda: _h[0]
    sys.modules["antenv.axon_hooks"] = mod
    antenv.axon_hooks = mod
    try:
        from trn_agent_boot.trn_boot import _ntff_profile_via_ctypes
        mod.set_axon_ntff_profile_hook(
            _ntff_profile_via_ctypes("/opt/axon/libaxon_pjrt.so")
        )
    except Exception:
        pass


def kernel(x, positions, w_q, w_k, w_v, w_o):
    global _BUILT, LAST_RESULTS
    from concourse.bass_utils import run_bass_kernel_spmd

    x = np.asarray(x)
    positions = np.asarray(positions)
    w_q = np.asarray(w_q, np.float32)
    w_k = np.asarray(w_k, np.float32)
    w_v = np.asarray(w_v, np.float32)
    w_o = np.asarray(w_o, np.float32)

    if _BUILT is None:
        _BUILT = _build()
    nc = _BUILT

    in_maps = _host_prep(x, positions, w_q, w_k, w_v, w_o)
    trace = os.environ.get("BASS_KERNEL_TRACE", "0") == "1"
    if trace:
        _ensure_ntff_hook()
    res = run_bass_kernel_spmd(
        nc, in_maps, core_ids=list(range(N_CORES)), trace=trace
    )
    LAST_RESULTS = res

    out = np.empty((T, HID), np.float32)
    for c in range(N_CORES):
        out[:, NQ * c:NQ * (c + 1)] = np.asarray(res.results[c]["outT"]).T
    return out.reshape(B, S, HID)


# revision 31
# speedup vs baseline: 1.0508x; 1.0508x over previous
"""GQA attention layer (B=2, S=2048, HID=4096, 32 Q heads / 8 KV heads, RoPE,
causal) on 8 TRN2 NeuronCores.

Strategy (tensor-parallel over heads):
  - core c owns Q heads 4c..4c+3 and KV head c (one full GQA group).
  - host pre-transposes x and weights into [128, kc, feat] layouts so every
    input loads with a handful of large DMAs and every on-chip matmul
    contracts over the partition axis with no on-chip transposes of x.
  - Q/K/V are SBUF-resident: projections write Q^T/K^T (RoPE fused into the
    PSUM eviction on DVE) and V (TensorE-transposed) straight into
    persistent SBUF tiles; attention reads them with zero DMA.
  - the PE stream is kept dense end-to-end: batch-0 attention heads are
    interleaved with the batch-1 projection groups (so the AllGather chain
    starts ~half-way into the projection phase), and batch-1 attention
    heads are interleaved with o_proj chunks of already-gathered blocks.
  - attention is emitted score-lookahead (s0,s1,c0,s2,c1,...) so the PE
    FIFO never waits on the ACT exp; causal diagonal 128-chunks use
    narrowed q-ranges (exact triangle FLOPs) with a single 128x128
    triangular mask multiply; softmax denominator accumulates in bf16 and
    becomes one ones[128x128] matmul broadcast + ACT copy + DVE divide.
  - ctx^T is AllGathered per 512-token block (8 chunks); the host
    concatenates the 8 per-core 512-column o_proj output slices.
"""

import os

os.environ.setdefault("NEURON_RT_DBG_RDH_CC", "0")

import numpy as np
import ml_dtypes

B, S, HID = 2, 2048, 4096
NH, NKV, D = 32, 8, 128
T = B * S            # 4096 flattened tokens
NQ = 512             # per-core q features (4 heads x 128)
P = 128
TOKB = 512           # token block (matmul moving free dim)
NB = T // TOKB       # 8 token blocks
KC = HID // P        # 32 contraction chunks for projections
QBS = S // TOKB      # 4 q blocks per batch
KTS = S // P         # 16 k chunks per batch
NHL = 4              # local Q heads per core
SCALE = 1.0 / float(np.sqrt(np.float32(D)))
N_CORES = 8

_BUILT = None
LAST_RESULTS = None


def _build():
    from contextlib import ExitStack

    import concourse.tile as tile
    from concourse import bacc, mybir

    f32 = mybir.dt.float32
    bf16 = mybir.dt.bfloat16
    Exp = mybir.ActivationFunctionType.Exp

    nc = bacc.Bacc(
        "TRN2",
        target_bir_lowering=False,
        debug=False,
        num_devices=N_CORES,
    )

    xT = nc.declare_dram_parameter("xT", [P, KC, T], bf16, isOutput=False)
    wqT = nc.declare_dram_parameter("wqT", [P, KC, NQ], bf16, isOutput=False)
    wkT = nc.declare_dram_parameter("wkT", [P, KC, D], bf16, isOutput=False)
    wvT = nc.declare_dram_parameter("wvT", [P, KC, D], bf16, isOutput=False)
    woT = nc.declare_dram_parameter("woT", [P, KC, NQ], bf16, isOutput=False)
    cosT = nc.declare_dram_parameter("cosT", [64, T], f32, isOutput=False)
    sinT = nc.declare_dram_parameter("sinT", [64, T], f32, isOutput=False)
    maskT = nc.declare_dram_parameter("maskT", [P, P], bf16, isOutput=False)
    outT = nc.declare_dram_parameter("outT", [NQ, T], f32, isOutput=True)

    XC = 8               # kc chunks per x DMA
    NXD = KC // XC       # 4 x DMAs per token block

    with tile.TileContext(nc) as tc, ExitStack() as gctx:
        ec = gctx.enter_context
        # ---- global pools (whole-kernel lifetime) ----
        dram = ec(tc.tile_pool(name="dram", bufs=1, space="DRAM"))
        const_pool = ec(tc.tile_pool(name="const_sb", bufs=1))
        qkv_pool = ec(tc.tile_pool(name="qkv_sb", bufs=1))
        # PSUM budget (8 banks): ps 3 + ctxp 2 + rsp {vps,rbp} 2 + opp 1
        ps_pool = ec(tc.tile_pool(name="ps", bufs=3, space="PSUM"))
        ctxp_pool = ec(tc.tile_pool(name="ctxp", bufs=2, space="PSUM"))
        rs_pool = ec(tc.tile_pool(name="rsp", bufs=1, space="PSUM"))
        op_pool = ec(tc.tile_pool(name="opp", bufs=1, space="PSUM"))
        # attention working pools (live through both halves)
        e_pool = ec(tc.tile_pool(name="e_sb", bufs=6))
        acc_pool = ec(tc.tile_pool(name="acc_sb", bufs=3))
        rbc_pool = ec(tc.tile_pool(name="rbc_sb", bufs=2))
        ctx_out_pool = ec(tc.tile_pool(name="ctx_sb", bufs=2))

        # token-chunked AllGather buffers (0.5 MB per rank per chunk)
        ag_in = [dram.tile([P, NHL * TOKB], bf16, name=f"ag_in{t}")
                 for t in range(NB)]
        ag_out = [
            dram.tile([N_CORES * P, NHL * TOKB], bf16, addr_space="Shared",
                      name=f"ag_out{t}")
            for t in range(NB)
        ]

        ones_sq = const_pool.tile([P, P], bf16, name="ones_sq")
        nc.vector.memset(ones_sq[:, :], 1.0)
        ident = const_pool.tile([P, P], bf16, name="ident")
        from concourse.masks import make_identity
        make_identity(nc, ident[:, :])
        tri_sb = const_pool.tile([P, P], bf16, name="tri_sb")
        nc.sync.dma_start(out=tri_sb[:, :], in_=maskT[:, :])

        # persistent Q^T / K^T / V tiles (SBUF-resident between phases)
        q_sb = qkv_pool.tile([P, NHL, T], bf16, name="q_sb")
        kt_sb = qkv_pool.tile([P, T], bf16, name="kt_sb")
        v_sb = [qkv_pool.tile([P, P], bf16, name=f"v_sb{g}")
                for g in range(T // P)]  # 32 chunks

        # ---------------- fine-grained filler pump ----------------
        # Filler work (projection groups / o_proj chunks) is emitted as
        # generators yielding after each matmul, so single filler MMs can
        # slot into the exp-latency bubbles inside attention heads.
        class Pump:
            def __init__(self):
                self.gens = []
                self.left = 0

            def add(self, gen, n_units):
                self.gens.append(gen)
                self.left += n_units

            def pump(self, n):
                done = 0
                while done < n and self.gens:
                    try:
                        next(self.gens[0])
                        done += 1
                    except StopIteration:
                        self.gens.pop(0)
                self.left -= done
                return done

            def drain(self):
                self.pump(1 << 30)

        # ---------------- attention head / AG emitters ----------------
        def attn_head(tb, h, ctxw, pump=None):
            b, qb = tb // QBS, tb % QBS
            qcols = slice(S * b + TOKB * qb, S * b + TOKB * (qb + 1))
            nkt = 4 * qb + 4
            qh = q_sb[:, h, qcols]
            acc = acc_pool.tile([P, TOKB], bf16, name="acc")
            ctxp = ctxp_pool.tile([P, TOKB], f32, name="ctxp")
            es = [None] * nkt

            def q_lo(kt):
                j = kt - 4 * qb
                return 0 if j < 0 else P * j

            def emit_score(kt):
                lo = q_lo(kt)
                sp = ps_pool.tile([P, TOKB], f32, name="ps")
                nc.tensor.matmul(
                    sp[:, lo:],
                    kt_sb[:, S * b + P * kt:S * b + P * (kt + 1)],
                    qh[:, lo:],
                    start=True, stop=True,
                )
                e = e_pool.tile([P, TOKB], bf16, name="e")
                nc.scalar.activation(e[:, lo:], sp[:, lo:], Exp, scale=SCALE)
                if kt - 4 * qb >= 0:
                    nc.vector.tensor_mul(
                        e[:, lo:lo + P], e[:, lo:lo + P], tri_sb[:, :]
                    )
                if kt == 0:
                    nc.vector.tensor_copy(acc[:, lo:], e[:, lo:])
                else:
                    nc.vector.tensor_add(acc[:, lo:], acc[:, lo:], e[:, lo:])
                es[kt] = e

            def emit_ctx(kt):
                lo = q_lo(kt)
                nc.tensor.matmul(
                    ctxp[:, lo:],
                    v_sb[16 * b + kt][:, :],
                    es[kt][:, lo:],
                    start=(kt == 0), stop=(kt == nkt - 1),
                )

            # score-lookahead emission: s0 s1 c0 s2 c1 ... c(n-1), with one
            # filler MM pumped per pair to cover the ACT exp latency
            emit_score(0)
            for kt in range(1, nkt):
                emit_score(kt)
                emit_ctx(kt - 1)
                if pump is not None:
                    pump.pump(1)
            emit_ctx(nkt - 1)

            # denominator: ones[128,128]^T @ acc = broadcast rowsum
            rbp = rs_pool.tile([P, TOKB], f32, name="rbp")
            nc.tensor.matmul(
                rbp[:, :], ones_sq[:, :], acc[:, :], start=True, stop=True
            )
            rbc = rbc_pool.tile([P, TOKB], f32, name="rbc")
            nc.vector.reciprocal(rbc[:, :], rbp[:, :])
            nc.vector.tensor_mul(
                ctxw[:, TOKB * h:TOKB * (h + 1)], ctxp[:, :], rbc[:, :]
            )
            # per-head store so the AllGather trigger only waits on head 3
            nc.sync.dma_start(
                out=ag_in[tb][:, TOKB * h:TOKB * (h + 1)],
                in_=ctxw[:, TOKB * h:TOKB * (h + 1)],
            )

        def emit_ag(tb, ctxw):
            nc.gpsimd.collective_compute(
                "AllGather",
                mybir.AluOpType.bypass,
                replica_groups=[list(range(N_CORES))],
                ins=[ag_in[tb][:, :].opt()],
                outs=[ag_out[tb][:, :].opt()],
            )

        # ================= Projections + batch-0 attention =================
        with ExitStack() as actx:
            aec = actx.enter_context
            w_pool = aec(tc.tile_pool(name="w_sb", bufs=1))
            xt_pool = aec(tc.tile_pool(name="xt_sb", bufs=5))
            rope_pool = aec(tc.tile_pool(name="rope_sb", bufs=1))
            rtmp_pool = aec(tc.tile_pool(name="rtmp_sb", bufs=1))
            vtmp_pool = aec(tc.tile_pool(name="vtmp_sb", bufs=2))

            # wq + x(block 0) first in the DMA queue: they gate the first MM
            wq_sb = w_pool.tile([P, KC, NQ], bf16, name="wq")
            nc.sync.dma_start(out=wq_sb[:, :, :], in_=wqT[:, :, :])
            wk_sb = w_pool.tile([P, KC, D], bf16, name="wk")
            wv_sb = w_pool.tile([P, KC, D], bf16, name="wv")
            # rope tables are identical for both batches: store one S-column
            # copy and index by within-batch offset
            cos_sb = rope_pool.tile([64, S], f32, name="cos_sb")
            sin_sb = rope_pool.tile([64, S], f32, name="sin_sb")

            def load_rest():
                nc.sync.dma_start(out=wk_sb[:, :, :], in_=wkT[:, :, :])
                nc.sync.dma_start(out=wv_sb[:, :, :], in_=wvT[:, :, :])
                nc.sync.dma_start(out=cos_sb[:, :], in_=cosT[:, 0:S])
                nc.sync.dma_start(out=sin_sb[:, :], in_=sinT[:, 0:S])

            def rope_evict(psum, dest, bcols):
                """psum [128(d), 512(tok)] f32 -> RoPE -> dest bf16 slices.
                bcols is the within-batch column slice [0, S)."""
                c_ap = cos_sb[:, bcols]
                s_ap = sin_sb[:, bcols]
                p0 = psum[0:64, :]
                p1 = psum[64:128, :]
                t0 = rtmp_pool.tile([64, TOKB], f32, name="t0")
                t1 = rtmp_pool.tile([64, TOKB], f32, name="t1")
                nc.vector.tensor_mul(t0[:, :], p0, c_ap)
                nc.vector.tensor_mul(t1[:, :], p1, s_ap)
                nc.vector.tensor_sub(dest[0], t0[:, :], t1[:, :])
                t2 = rtmp_pool.tile([64, TOKB], f32, name="t2")
                t3 = rtmp_pool.tile([64, TOKB], f32, name="t3")
                nc.vector.tensor_mul(t2[:, :], p0, s_ap)
                nc.vector.tensor_mul(t3[:, :], p1, c_ap)
                nc.vector.tensor_add(dest[1], t2[:, :], t3[:, :])

            def proj_gens(nb):
                """(generator, n_matmul_units) emitters for token block nb;
                generators yield after each MM so they can pump-fill."""
                cols = slice(TOKB * nb, TOKB * (nb + 1))
                bcols = slice(TOKB * (nb % QBS), TOKB * (nb % QBS + 1))
                xts = []

                def load_x():
                    for xd in range(NXD):
                        xt_t = xt_pool.tile([P, XC, TOKB], bf16, name="xt")
                        nc.sync.dma_start(
                            out=xt_t[:, :, :],
                            in_=xT[:, XC * xd:XC * (xd + 1), cols],
                        )
                        xts.append(xt_t)
                    return
                    yield  # pragma: no cover

                def xr(kc):
                    return xts[kc // XC][:, kc % XC, :]

                def q_group(m):
                    psum = ps_pool.tile([P, TOKB], f32, name="ps")
                    for kc in range(KC):
                        nc.tensor.matmul(
                            psum[:, :],
                            wq_sb[:, kc, P * m:P * (m + 1)],
                            xr(kc),
                            start=(kc == 0), stop=(kc == KC - 1),
                        )
                        yield
                    rope_evict(
                        psum,
                        (q_sb[0:64, m, cols], q_sb[64:128, m, cols]),
                        bcols,
                    )

                def k_group():
                    psum = ps_pool.tile([P, TOKB], f32, name="ps")
                    for kc in range(KC):
                        nc.tensor.matmul(
                            psum[:, :], wk_sb[:, kc, :], xr(kc),
                            start=(kc == 0), stop=(kc == KC - 1),
                        )
                        yield
                    rope_evict(
                        psum, (kt_sb[0:64, cols], kt_sb[64:128, cols]), bcols
                    )

                def v_group():
                    psum = ps_pool.tile([P, TOKB], f32, name="ps")
                    for kc in range(KC):
                        nc.tensor.matmul(
                            psum[:, :], wv_sb[:, kc, :], xr(kc),
                            start=(kc == 0), stop=(kc == KC - 1),
                        )
                        yield
                    vtmp = vtmp_pool.tile([P, TOKB], bf16, name="vtmp")
                    nc.scalar.copy(vtmp[:, :], psum[:, :])
                    for c in range(4):
                        vps = rs_pool.tile([P, P], bf16, name="vps")
                        nc.tensor.transpose(
                            vps[:, :], vtmp[:, P * c:P * (c + 1)], ident[:, :]
                        )
                        yield
                        nc.scalar.copy(v_sb[4 * nb + c][:, :], vps[:, :])

                return ([(load_x(), 0)]
                        + [(q_group(m), KC) for m in range(4)]
                        + [(k_group(), KC), (v_group(), KC + 4)])

            # part 1: projections for batch 0 (blocks 0..3), dense.
            # x(0) DMA right after wq; remaining weights + rope tables after.
            for nb in range(4):
                gens = proj_gens(nb)
                for _ in gens[0][0]:
                    pass
                if nb == 0:
                    load_rest()
                for g, _ in gens[1:]:
                    for _ in g:
                        pass

            # part 2: batch-0 attention with batch-1 projection MMs pumped
            # into the exp-latency bubbles
            pmp = Pump()
            for nb in range(4, 8):
                for g, u in proj_gens(nb):
                    pmp.add(g, u)
            slot = 0
            for tb in range(4):
                ctxw = ctx_out_pool.tile([P, NHL * TOKB], bf16, name="ctxw")
                for h in range(NHL):
                    attn_head(tb, h, ctxw, pump=pmp)
                    quota = -(-pmp.left // (16 - slot))
                    pmp.pump(quota)
                    slot += 1
                emit_ag(tb, ctxw)
            pmp.drain()

        # ============ batch-1 attention + interleaved o_proj ============
        with ExitStack() as bctx:
            bec = bctx.enter_context
            wo_pool = bec(tc.tile_pool(name="wo_sb", bufs=1))
            cx_pool = bec(tc.tile_pool(name="cx_sb", bufs=2))
            o_out_pool = bec(tc.tile_pool(name="o_sb", bufs=3))

            wo_sb = wo_pool.tile([P, KC, NQ], bf16, name="wo")
            nc.sync.dma_start(out=wo_sb[:, :, :], in_=woT[:, :, :])

            cx_tiles = {}

            def load_cx(t):
                # ag_out rows 128r+d; SBUF chunk index fc = 4r+h matches the
                # natural global-head feature order of woT.
                cx = cx_pool.tile([P, N_CORES, NHL, TOKB], bf16, name="cx")
                nc.sync.dma_start(
                    out=cx[:, :, :, :],
                    in_=ag_out[t][:, :].rearrange(
                        "(r d) (h c) -> d r h c", r=N_CORES, h=NHL
                    ),
                )
                cx_tiles[t] = cx

            def oproj_chunk(t, ot):
                cols = slice(TOKB * t, TOKB * (t + 1))
                if t not in cx_tiles:
                    load_cx(t)
                cx = cx_tiles[t]
                psum = op_pool.tile([P, TOKB], f32, name="ops")
                for fc in range(KC):
                    nc.tensor.matmul(
                        psum[:, :],
                        wo_sb[:, fc, P * ot:P * (ot + 1)],
                        cx[:, fc // NHL, fc % NHL, :],
                        start=(fc == 0), stop=(fc == KC - 1),
                    )
                    yield
                ov = o_out_pool.tile([P, TOKB], f32, name="ov")
                nc.vector.tensor_copy(ov[:, :], psum[:, :])
                nc.sync.dma_start(
                    out=outT[P * ot:P * (ot + 1), cols], in_=ov[:, :]
                )

            load_cx(0)
            load_cx(1)
            pm3 = Pump()
            for t in range(NB - 2):
                for ot in range(NHL):
                    pm3.add(oproj_chunk(t, ot), KC)
            cx_sched = {4: 2, 7: 3, 9: 4, 12: 5}
            slot = 0
            for tb in range(4, NB):
                ctxw = ctx_out_pool.tile([P, NHL * TOKB], bf16, name="ctxw")
                for h in range(NHL):
                    if slot in cx_sched:
                        load_cx(cx_sched[slot])
                    attn_head(tb, h, ctxw, pump=pm3)
                    quota = -(-pm3.left // (16 - slot))
                    pm3.pump(quota)
                    slot += 1
                emit_ag(tb, ctxw)
            pm3.drain()

            load_cx(NB - 2)
            for _ in oproj_chunk(NB - 2, 0):
                pass
            for ot in range(1, NHL):
                for _ in oproj_chunk(NB - 2, ot):
                    pass

            # last block: stream cx in rank-pair quarters, accumulate the 4
            # output tiles in parallel so o_proj overlaps the cx DMAs
            t = NB - 1
            cols = slice(TOKB * t, TOKB * (t + 1))
            cxl = cx_pool.tile([P, N_CORES, NHL, TOKB], bf16, name="cx")
            for rr in range(4):
                nc.sync.dma_start(
                    out=cxl[:, 2 * rr:2 * (rr + 1), :, :],
                    in_=ag_out[t][2 * rr * P:2 * (rr + 1) * P, :].rearrange(
                        "(r d) (h c) -> d r h c", r=2, h=NHL
                    ),
                )
            psums = [op_pool.tile([P, TOKB], f32, name="ops")] + [
                ps_pool.tile([P, TOKB], f32, name="ps") for _ in range(3)
            ]
            for rr in range(4):
                for ot in range(NHL):
                    for fi in range(8):
                        fc = 8 * rr + fi
                        nc.tensor.matmul(
                            psums[ot][:, :],
                            wo_sb[:, fc, P * ot:P * (ot + 1)],
                            cxl[:, fc // NHL, fc % NHL, :],
                            start=(rr == 0 and fi == 0),
                            stop=(rr == 3 and fi == 7),
                        )
            for ot in range(NHL):
                ov = o_out_pool.tile([P, TOKB], f32, name="ov")
                nc.vector.tensor_copy(ov[:, :], psums[ot][:, :])
                nc.sync.dma_start(
                    out=outT[P * ot:P * (ot + 1), cols], in_=ov[:, :]
                )

    nc.finalize()
    return nc


def _host_prep(x, positions, w_q, w_k, w_v, w_o):
    bf = ml_dtypes.bfloat16

    def feat_major(w):
        # [F, HID] -> [128, KC, F]  (hid = 128*kc + p)
        F = w.shape[0]
        return np.ascontiguousarray(
            w.T.reshape(KC, P, F).transpose(1, 0, 2)
        ).astype(bf)

    xT = feat_major(x.reshape(T, HID))          # [128, 32, T]

    half = D // 2
    inv_freq = 1.0 / (10000.0 ** (np.arange(half, dtype=np.float32) / half))
    freqs = np.outer(np.asarray(positions, np.float32), inv_freq)  # [S, 64]
    cosT1 = np.cos(freqs).T.astype(np.float32)  # [64, S]
    sinT1 = np.sin(freqs).T.astype(np.float32)
    cosT = np.ascontiguousarray(np.concatenate([cosT1] * B, axis=1))
    sinT = np.ascontiguousarray(np.concatenate([sinT1] * B, axis=1))

    dk = np.arange(P, dtype=np.int64)[:, None]
    dq = np.arange(P, dtype=np.int64)[None, :]
    maskT = np.ascontiguousarray((dk <= dq).astype(np.float32)).astype(bf)

    in_maps = []
    for c in range(N_CORES):
        in_maps.append({
            "xT": xT,
            "wqT": feat_major(w_q[NQ * c:NQ * (c + 1), :]),
            "wkT": feat_major(w_k[D * c:D * (c + 1), :]),
            "wvT": feat_major(w_v[D * c:D * (c + 1), :]),
            "woT": feat_major(w_o[NQ * c:NQ * (c + 1), :]),
            "cosT": cosT, "sinT": sinT, "maskT": maskT,
        })
    return in_maps


def _ensure_ntff_hook():
    """The agent image's antenv lacks axon_hooks; synthesize it so
    run_bass_kernel_spmd(trace=True) can capture NTFF profiles."""
    import sys
    import types
    try:
        from antenv.axon_hooks import get_axon_ntff_profile_hook  # noqa: F401
        return
    except ImportError:
        pass
    import antenv
    mod = types.ModuleType("antenv.axon_hooks")
    _h = [None]
    mod.set_axon_ntff_profile_hook = lambda h: _h.__setitem__(0, h)
    mod.get_axon_ntff_profile_hook = lambda: _h[0]
    sys.modules["antenv.axon_hooks"] = mod
    antenv.axon_hooks = mod
    try:
        from trn_agent_boot.trn_boot import _ntff_profile_via_ctypes
        mod.set_axon_ntff_profile_hook(
            _ntff_profile_via_ctypes("/opt/axon/libaxon_pjrt.so")
        )
    except Exception:
        pass


def kernel(x, positions, w_q, w_k, w_v, w_o):
    global _BUILT, LAST_RESULTS
    from concourse.bass_utils import run_bass_kernel_spmd

    x = np.asarray(x)
    positions = np.asarray(positions)
    w_q = np.asarray(w_q, np.float32)
    w_k = np.asarray(w_k, np.float32)
    w_v = np.asarray(w_v, np.float32)
    w_o = np.asarray(w_o, np.float32)

    if _BUILT is None:
        _BUILT = _build()
    nc = _BUILT

    in_maps = _host_prep(x, positions, w_q, w_k, w_v, w_o)
    trace = os.environ.get("BASS_KERNEL_TRACE", "0") == "1"
    if trace:
        _ensure_ntff_hook()
    res = run_bass_kernel_spmd(
        nc, in_maps, core_ids=list(range(N_CORES)), trace=trace
    )
    LAST_RESULTS = res

    out = np.empty((T, HID), np.float32)
    for c in range(N_CORES):
        out[:, NQ * c:NQ * (c + 1)] = np.asarray(res.results[c]["outT"]).T
    return out.reshape(B, S, HID)
